# revision 61
# baseline (speedup 1.0000x reference)
"""Causal self-attention Trainium2 kernel.

B=4, T=2048, C=1024, H=16 heads, D=64. 8 NeuronCores, tensor-parallel over
heads: core c owns heads {2c, 2c+1}. Host pre-transposes x to xT [C, B*T],
column-shards W_attn / row-shards W_proj, sums the 8 bf16 partial outputs.

Device kernel (per core, SPMD). The attention stream is paced by the exp
chain on ACT (~0.9us per key chunk), so every other engine's work is
emission-scheduled around it:
  qkv:  q,k rows [128ch, tok] = W.T @ xT (bf16, K=C in 8 chunks), bias on
        eviction; v computed token-major directly (out[tok, vch] with
        lhsT=x chunk) into per-key-chunk V tiles with a 65th ones column —
        no PE transpose, v bias folded into the yt eviction instead.
        Blocks are split into ~1us steps, each step evicting the PREVIOUS
        step's psum (so the DVE never head-of-line blocks on a fresh
        matmul), and a proportional pacer feeds steps into the attention
        key loops as PE bubble filler.
  attn: S^T[keys, queries] per key chunk via matmul(lhsT=kT, rhs=qT, bf16,
        K=64), exact causal trim. exp on ACT over both heads at once;
        causal mask via per-head gpsimd affine_select on the diagonal
        window. P@V in y-form: per 128-query subchunk,
        matmul(out[128q, 65], lhsT=P^T tile, rhs=[V|1]) accumulated over
        key chunks in psum; the 65th column accumulates the softmax
        denominator row-aligned with y. P@V of chunk j-1 rides behind S of
        chunk j. Normalize rides the psum eviction (tensor_scalar mult by
        recip(denominator), per-partition scalar), spread across the key
        loop as each query subchunk's accumulation completes; y chunks are
        PE-transposed (bf16 identity, 1cyc/row) to d-major.
  proj: partial out[tokens, C] = yT.T @ W_proj (bf16), interleaved into the
        next block's key loop as mm/evict half-steps; evictions split
        DVE/Pool; bf16 partials DMA'd out and summed on host.
"""

import sys

sys.path.insert(0, "/opt/trn_rl_repo")

from contextlib import ExitStack

import numpy as np

import concourse.bass as bass
import concourse.mybir as mybir
import concourse.tile as tile
from concourse import bacc
from concourse.bass_utils import run_bass_kernel_spmd
from concourse.masks import make_identity

F32 = mybir.dt.float32
BF16 = mybir.dt.bfloat16
AF = mybir.ActivationFunctionType

B, T, C, H, D = 4, 2048, 1024, 16, 64
NCORES = 8
HPC = H // NCORES  # heads per core = 2
TOK = B * T  # 8192
QKVC = HPC * D  # per-core channels per q/k/v = 128
TB = 256  # token block for the qkv phase
NBB = T // TB  # qkv token blocks per batch = 8
QB = 512  # query block for attention
NQC = QB // 128  # 128-query subchunks per block = 4
NKC = T // 128  # key chunks per batch = 16
SCALE = 1.0 / 8.0  # 1/sqrt(D)


def build_program():
    nc = bacc.Bacc(
        "TRN2",
        target_bir_lowering=False,
        debug=False,
        num_devices=NCORES,
    )
    xt_d = nc.dram_tensor("xt", [C, TOK], BF16, kind="ExternalInput").ap()
    wqkv_d = nc.dram_tensor("wqkv", [C, 3 * QKVC], BF16, kind="ExternalInput").ap()
    bqkv_d = nc.dram_tensor("bqkv", [3 * QKVC], F32, kind="ExternalInput").ap()
    wproj_d = nc.dram_tensor("wproj", [QKVC, C], BF16, kind="ExternalInput").ap()
    outp_d = nc.dram_tensor("outp", [TOK, C], BF16, kind="ExternalOutput").ap()

    with tile.TileContext(nc) as tc:
        with ExitStack() as ctx, nc.allow_low_precision(reason="bf16 matmul inputs"):
            _body(ctx, tc, xt_d, wqkv_d, bqkv_d, wproj_d, outp_d)
    nc.compile()
    return nc


class _Kern:
    def __init__(self, ctx, tc, xt_d, wqkv_d, bqkv_d, wproj_d, outp_d):
        nc = tc.nc
        self.nc = nc
        self.tc = tc
        self.outp_d = outp_d

        self.const = ctx.enter_context(tc.tile_pool(name="const", bufs=1))
        self.persist = ctx.enter_context(tc.tile_pool(name="persist", bufs=1))
        self.xt_pool = ctx.enter_context(tc.tile_pool(name="xt", bufs=8))
        self.pt_pool = ctx.enter_context(tc.tile_pool(name="pt", bufs=6))
        self.ytmp_pool = ctx.enter_context(tc.tile_pool(name="ytmp", bufs=6))
        self.yt_pool = ctx.enter_context(tc.tile_pool(name="yt", bufs=2))
        self.out_pool = ctx.enter_context(tc.tile_pool(name="osb", bufs=6))
        self.small_pool = ctx.enter_context(tc.tile_pool(name="small", bufs=8))

        self.ps_s = ctx.enter_context(tc.tile_pool(name="ps_s", bufs=2, space="PSUM"))
        self.ps_y = ctx.enter_context(tc.tile_pool(name="ps_y", bufs=1, space="PSUM"))
        self.ps_mm = ctx.enter_context(tc.tile_pool(name="ps_mm", bufs=2, space="PSUM"))

        # --- constants ---
        c = self.const
        # weight loads ride the ACT HWDGE ring (nc.scalar) so they don't
        # serialize with the xt streaming loads on the SP ring; the first
        # K-chunk is split out so the PE can start quickly
        self.wqkv_s = c.tile([128, 8, 3 * QKVC], BF16, tag="wqkv", name="wqkv_s")
        wqkv_r = wqkv_d.rearrange("(kc p) m -> p kc m", p=128)
        nc.scalar.dma_start(self.wqkv_s[:, 0:1, :], wqkv_r[:, 0:1, :])
        nc.scalar.dma_start(self.wqkv_s[:, 1:8, :], wqkv_r[:, 1:8, :])
        self.bqkv_s = c.tile([128, 3], F32, tag="bqkv", name="bqkv_s")
        nc.scalar.dma_start(self.bqkv_s[:], bqkv_d.rearrange("(m p) -> p m", p=128))
        self.wproj_s = c.tile([128, C], BF16, tag="wproj", name="wproj_s")
        nc.scalar.dma_start(self.wproj_s[:], wproj_d[:])
        self.identB = c.tile([128, 128], BF16, tag="identB", name="identB")
        make_identity(nc, self.identB[:])
        # causal mask for the diagonal 128x128 window: within the window the
        # global (query - key) offset is always col - partition, so one
        # lower-triangle keep-mask (col >= partition) serves every diagonal
        # chunk; applied as a bf16 multiply on DVE (2x mode, ~127ns)
        self.tri = c.tile([128, 128], BF16, tag="tri", name="tri")
        nc.gpsimd.memset(self.tri[:], 1.0)
        nc.gpsimd.affine_select(
            out=self.tri[:],
            in_=self.tri[:],
            base=0,
            channel_multiplier=-1,
            pattern=[[1, 128]],
            compare_op=mybir.AluOpType.is_ge,
            fill=0.0,
        )

        # persistent activations
        self.qT = self.persist.tile([128, TOK], BF16, tag="qT", name="qT")
        self.kT = self.persist.tile([128, TOK], BF16, tag="kT", name="kT")
        # token-major V per key chunk (cols 0:D per head) + ones column
        # (col D): the y-form P@V matmul then accumulates the softmax
        # denominator on psum column D, row-aligned with y
        self.vones = self.persist.tile(
            [128, B, NKC, HPC, D + 1], BF16, tag="vones", name="vones"
        )
        nc.gpsimd.memset(self.vones[:, :, :, :, D : D + 1], 1.0)
        self.xt_r = xt_d.rearrange("(kc p) t -> p kc t", p=128)

    def qkv_block_steps(self, b, nb):
        """QKV for token block nb of batch b as ~1us emission steps. Each
        step evicts the previous step's psum (already compute-complete by
        then) so the DVE never blocks waiting on a fresh matmul chain."""
        nc = self.nc
        n = b * NBB + nb
        st = {}

        def load():
            xt_t = self.xt_pool.tile([128, 8, TB], BF16, tag="xt", name=f"xt{n}")
            if n == 0:
                # cold start: peel off K-chunk 0 so the PE can start sooner
                # without paying 8 serial HWDGE issues
                nc.sync.dma_start(
                    xt_t[:, 0, :], self.xt_r[:, 0, n * TB : (n + 1) * TB]
                )
                nc.sync.dma_start(
                    xt_t[:, 1:8, :], self.xt_r[:, 1:8, n * TB : (n + 1) * TB]
                )
            else:
                nc.sync.dma_start(xt_t[:], self.xt_r[:, :, n * TB : (n + 1) * TB])
            st["xt"] = xt_t

        def mm_qk(m):
            ps = self.ps_mm.tile([128, TB], F32, tag="mm", name=f"qkp{n}_{m}")
            for kc in range(8):
                nc.tensor.matmul(
                    ps[:],
                    self.wqkv_s[:, kc, m * 128 : (m + 1) * 128],
                    st["xt"][:, kc, :],
                    start=(kc == 0),
                    stop=(kc == 7),
                )
            st[m] = ps

        def ev_qk(m):
            dst = (self.qT if m == 0 else self.kT)[:, n * TB : (n + 1) * TB]
            nc.vector.tensor_scalar_add(dst, st.pop(m)[:], self.bqkv_s[:, m : m + 1])

        def mm_v(tt):
            # v token-major directly: out[tok, vch] = x_chunk.T @ Wv_chunk;
            # both 128-token sub-chunks share one psum bank (single start per
            # bank rule: only tt=0 kc=0 starts) and evict in one DVE op
            if tt == 0:
                st["vps"] = self.ps_mm.tile(
                    [128, 2, QKVC], F32, tag="mm", name=f"vp{n}"
                )
            ps = st["vps"]
            for kc in range(8):
                nc.tensor.matmul(
                    ps[:, tt, :],
                    st["xt"][:, kc, tt * 128 : (tt + 1) * 128],
                    self.wqkv_s[:, kc, 2 * QKVC : 3 * QKVC],
                    start=(kc == 0 and tt == 0),
                    stop=(kc == 7),
                    skip_group_check=True,
                )

        def ev_v():
            kc = 2 * nb
            nc.vector.tensor_copy(
                self.vones[:, b, kc : kc + 2, :, 0:D],
                st.pop("vps")[:].rearrange("p tt (h d) -> p tt h d", h=HPC),
            )

        return load, [
            lambda: mm_qk(0),
            lambda: (ev_qk(0), mm_qk(1)),
            lambda: (ev_qk(1), mm_v(0)),
            lambda: mm_v(1),
            lambda: ev_v(),
        ]

    def _pv(self, b, qb, j, d, pt, psy):
        """y-form P@V for key chunk j: per (head, live query subchunk),
        out[128q, 65] += P^T_tile.T @ [V|1]."""
        nc = self.nc
        for h in range(HPC):
            for qc in range(max(d, 0), NQC):
                # exactly ONE start=True per psum bank: psum zeroing is
                # 2KB-region granular, so a start for qc>0 would re-mark the
                # whole bank pending-zero and clobber the other accumulators.
                # qc>0 groups accumulate onto pending-zero bytes (read as 0).
                nc.tensor.matmul(
                    psy[:, h, qc * 65 : qc * 65 + 65],
                    pt[:, h, qc * 128 : (qc + 1) * 128],
                    self.vones[:, b, j, h, :],
                    start=(j == 0 and qc == 0),
                    stop=(j == 4 * qb + qc),
                    skip_group_check=True,
                )

    def proj_tt(self, b, qb, ytT, tt, osbs):
        """Unstaggered projection of one 128-token chunk (kernel tail)."""
        nc = self.nc
        q0 = b * T + qb * QB
        pair = tt // 2
        if pair not in osbs:
            osbs[pair] = self.out_pool.tile(
                [128, 2, C], BF16, tag="osb", name=f"ot{b}_{qb}_{pair}"
            )
        pos = []
        for ncol in range(C // 512):
            po = self.ps_mm.tile([128, 512], F32, tag="mm", name=f"pot{b}_{qb}_{tt}_{ncol}")
            nc.tensor.matmul(
                po[:],
                ytT[:, tt * 128 : (tt + 1) * 128],
                self.wproj_s[:, ncol * 512 : (ncol + 1) * 512],
                start=True,
                stop=True,
            )
            pos.append(po)
        for ncol in range(C // 512):
            nc.vector.tensor_copy(
                osbs[pair][:, tt % 2, ncol * 512 : (ncol + 1) * 512], pos[ncol][:]
            )
        if tt % 2 == 1:
            r0 = q0 + (tt - 1) * 128
            dst = self.outp_d[r0 : r0 + 256, :].rearrange("(two p) c -> p two c", p=128)
            nc.sync.dma_start(dst, osbs.pop(pair)[:])

    def attn_block(self, s, b, qb, proj_halves, pacer, tail_cb=None):
        """Attention for query block qb (QB queries) of batch b.

        proj_halves: deque of (slot, thunk) projection half-steps from
        previous blocks; pacer(): emits qkv filler steps. Both interleave
        into the key loop as ready PE work behind the ACT-paced exp chain.
        Halves may spill one slot further, but anything two slots old must
        drain before this slot's ytT reuses its ring buffer.
        """
        nc = self.nc
        while proj_halves and proj_halves[0][0] <= s - 2:
            proj_halves.pop(0)[1]()
        q0 = b * T + qb * QB
        nj = (qb + 1) * NQC  # key chunks attended by this block
        psy = self.ps_y.tile([128, HPC, 512], F32, tag="y", name=f"psy{b}_{qb}")
        ytT = self.yt_pool.tile([128, QB], BF16, tag="yt", name=f"yt{b}_{qb}")
        st = {}

        def y_evict(qc):
            # qc's accumulation is complete (stop chunk was nj-4+qc, two
            # iterations ago): normalize on evict, then transpose to d-major
            rec = self.small_pool.tile(
                [128, HPC, 1], F32, tag="rec", name=f"rec{b}_{qb}_{qc}"
            )
            den = psy[:, :, qc * 65 + 64 : qc * 65 + 65]
            nc.vector.reciprocal(rec[:, :, :], den)
            ytmp = self.ytmp_pool.tile(
                [128, 128], BF16, tag="ytmp", name=f"ym{b}_{qb}_{qc}"
            )
            for h in range(HPC):
                nc.vector.tensor_scalar_mul(
                    ytmp[:, h * D : (h + 1) * D],
                    psy[:, h, qc * 65 : qc * 65 + 64],
                    rec[:, h, :],
                )
            pst = self.ps_mm.tile([128, 128], BF16, tag="mm", name=f"ytr{b}_{qb}_{qc}")
            nc.tensor.transpose(pst[:], ytmp[:], self.identB[:])
            st[qc] = pst

        def yt_finish(qc):
            # d-major eviction; v bias rides here: partitions are the (h,d)
            # v channels, and y_norm + b_v is exact post-normalization
            nc.vector.tensor_scalar_add(
                ytT[:, qc * 128 : (qc + 1) * 128], st.pop(qc)[:], self.bqkv_s[:, 2:3]
            )
            if tail_cb is not None:
                tail_cb(qc, ytT)

        prevq = []
        for j in range(nj):  # key chunks of 128
            k0 = b * T + j * 128
            # filler first: a wait-prone instruction stalls everything behind
            # it in the in-order PE queue, so ready work must precede it
            pacer()
            # exact causal trim: for a diagonal chunk at offset d, queries
            # below 128*d attend to no key in this chunk
            d = j - (nj - 4)
            f0 = 128 * d if d > 0 else 0
            ps2 = self.ps_s.tile([128, HPC, QB], F32, tag="s2", name=f"s{b}_{qb}_{j}")
            for h in range(HPC):
                nc.tensor.matmul(
                    ps2[:, h, f0:QB],
                    self.kT[h * D : (h + 1) * D, k0 : k0 + 128],
                    self.qT[h * D : (h + 1) * D, q0 + f0 : q0 + QB],
                    start=True,
                    stop=True,
                )
            pt = self.pt_pool.tile([128, HPC, QB], BF16, tag="pt", name=f"pt{b}_{qb}_{j}")
            nc.scalar.activation(pt[:, :, f0:QB], ps2[:, :, f0:QB], AF.Exp, scale=SCALE)
            if d >= 0:
                # mask only the 128-column window straddling the diagonal,
                # per head so the first P@V matmul isn't gated on both.
                # Pool (SBUF-only there, which is legal): keeps the DVE free
                # for psum evictions, which only DVE can do.
                for h in range(HPC):
                    nc.gpsimd.tensor_mul(
                        pt[:, h, f0 : f0 + 128],
                        pt[:, h, f0 : f0 + 128],
                        self.tri[:],
                    )
            # two-chunk software pipeline: P@V of chunk j-2 goes on the PE
            # behind S of chunk j, so the PE never waits on the exp chain
            if len(prevq) == 2:
                pr = prevq.pop(0)
                self._pv(b, qb, pr[0], pr[1], pr[2], psy)
            # spread the y eviction pipeline: qc's evict two iterations after
            # its stop chunk, the d-major eviction one more later
            if nj - 2 <= j < nj - 2 + NQC and j - (nj - 2) in range(NQC):
                y_evict(j - (nj - 2))
            if nj - 1 <= j and j - (nj - 1) in st:
                yt_finish(j - (nj - 1))
            if proj_halves:
                proj_halves.pop(0)[1]()
            prevq.append((j, d, pt))
        for pr in prevq:
            self._pv(b, qb, pr[0], pr[1], pr[2], psy)
        # in-loop iterations evicted qc=0,1 and finished qc=0; drain the rest
        for qc in range(2, NQC):
            y_evict(qc)
            yt_finish(qc - 1)
        yt_finish(NQC - 1)
        return ytT

    def proj_halves(self, b, qb, ytT):
        """Projection + output DMA per 128-token chunk, as mm/evict
        half-steps so each eviction lands an iteration after its matmul.
        Evictions alternate DVE/Pool; output DMAs ride the Pool SWDGE ring
        (two token chunks per DMA) so they never block xt loads on SP."""
        q0 = b * T + qb * QB
        halves = []
        st = {}

        def mm(tt, ncol):
            po = self.ps_mm.tile([128, 512], F32, tag="mm", name=f"po{b}_{qb}_{tt}_{ncol}")
            self.nc.tensor.matmul(
                po[:],
                ytT[:, tt * 128 : (tt + 1) * 128],
                self.wproj_s[:, ncol * 512 : (ncol + 1) * 512],
                start=True,
                stop=True,
            )
            st[(tt, ncol)] = po

        def ev(tt, ncol):
            pair = tt // 2
            if (pair, "osb") not in st:
                st[(pair, "osb")] = self.out_pool.tile(
                    [128, 2, C], BF16, tag="osb", name=f"o{b}_{qb}_{pair}"
                )
            osb = st[(pair, "osb")]
            # DVE: psum reads are illegal on GPSIMD
            self.nc.vector.tensor_copy(
                osb[:, tt % 2, ncol * 512 : (ncol + 1) * 512], st.pop((tt, ncol))[:]
            )
            if ncol == 1 and tt % 2 == 1:
                r0 = q0 + (tt - 1) * 128
                dst = self.outp_d[r0 : r0 + 256, :].rearrange(
                    "(two p) c -> p two c", p=128
                )
                self.nc.sync.dma_start(dst, st.pop((pair, "osb"))[:])

        # chain so exactly one po is in flight and each eviction lands one
        # iteration after its matmul: [mm0], [ev0, mm1], [ev1, mm2], ...
        units = [(tt, ncol) for tt in range(NQC) for ncol in range(2)]
        halves.append(lambda: mm(*units[0]))
        for i in range(1, len(units)):
            halves.append(lambda i=i: (ev(*units[i - 1]), mm(*units[i])))
        halves.append(lambda: ev(*units[-1]))
        return halves


def _body(ctx, tc, xt_d, wqkv_d, bqkv_d, wproj_d, outp_d):
    k = _Kern(ctx, tc, xt_d, wqkv_d, bqkv_d, wproj_d, outp_d)
    # qkv emission is step-granular: each token block is 5 steps of ~1us PE
    # work. A proportional pacer feeds steps into the ACT-paced attention key
    # loops so the PE never idles behind the exp chain; a per-slot `need`
    # check densely drains whatever the attended keys/queries require.
    steps = []  # (block_index, thunk)
    loads = []  # per-block xt DMA, prefetched ~2 blocks ahead of compute
    for b in range(B):
        for nb in range(NBB):
            ld, block_steps = k.qkv_block_steps(b, nb)
            loads.append(ld)
            for t in block_steps:
                steps.append((b * NBB + nb, t))
    si = 0  # next compute step to emit
    li = 0  # next load to emit

    def prefetch(upto_block):
        nonlocal li
        while li < len(loads) and li <= upto_block:
            loads[li]()
            li += 1

    def blocks_done():
        return steps[si][0] if si < len(steps) else B * NBB

    nslots = B * (T // QB)
    # JIT pacing against the need curve: block n must be fully emitted
    # before the first slot that attends to its tokens. Ramp linearly
    # across each slot's iterations (plus a small lookahead for DMA
    # latency) so filler lands INSIDE the ACT-paced attention stretches
    # instead of as dense pre-slot bursts that leave the PE idle later.
    spb = 5  # steps per block
    need_list = [
        (s // (T // QB)) * NBB + 2 * (s % (T // QB)) + 2 for s in range(nslots)
    ] + [B * NBB]
    targets = []
    for s in range(nslots):
        nj = 4 * ((s % (T // QB)) + 1)
        st0, st1 = spb * need_list[s], spb * need_list[s + 1]
        for jj in range(nj):
            targets.append(st0 + (st1 - st0) * (jj + 1) / nj)
    LOOK = 2
    state = {"it": 0}

    def pacer():
        nonlocal si
        tgt = targets[min(state["it"] + LOOK, len(targets) - 1)]
        state["it"] += 1
        while si < len(steps) and si < tgt:
            prefetch(steps[si][0] + 5)
            steps[si][1]()
            si += 1

    pending = []
    for s in range(nslots):
        b, qb = s // (T // QB), s % (T // QB)
        # attention (b, qb) touches keys/values up to chunk 4qb+3 and queries
        # up to token (qb+1)*QB of batch b: token blocks 0..2qb+1 of batch b
        need = need_list[s]
        while si < len(steps) and blocks_done() < need:
            prefetch(steps[si][0] + 5)
            steps[si][1]()
            si += 1
        if s == nslots - 1:
            # last slot: no later key loop to ride in; emit each token
            # chunk's projection as soon as its yt lands
            osbs = {}
            ytT = k.attn_block(
                s, b, qb, pending, pacer,
                tail_cb=lambda qc, yt: k.proj_tt(b, qb, yt, qc, osbs),
            )
            pending = []
        else:
            ytT = k.attn_block(s, b, qb, pending, pacer)
            pending.extend((s, t) for t in k.proj_halves(b, qb, ytT))
    for _, t in pending:
        t()
    while si < len(steps):
        prefetch(steps[si][0] + 5)
        steps[si][1]()
        si += 1


_CACHED_NC = None


def _get_nc():
    global _CACHED_NC
    if _CACHED_NC is None:
        _CACHED_NC = build_program()
    return _CACHED_NC


def make_in_maps(x, W_attn, b_attn, W_proj):
    x = np.ascontiguousarray(np.asarray(x, dtype=np.float32))
    W_attn = np.asarray(W_attn, dtype=np.float32)
    b_attn = np.asarray(b_attn, dtype=np.float32)
    W_proj = np.asarray(W_proj, dtype=np.float32)
    import ml_dtypes

    xt = np.ascontiguousarray(x.reshape(TOK, C).T.astype(ml_dtypes.bfloat16))
    in_maps = []
    for c in range(NCORES):
        s = c * QKVC
        wq = W_attn[:, s : s + QKVC]
        wk = W_attn[:, C + s : C + s + QKVC]
        wv = W_attn[:, 2 * C + s : 2 * C + s + QKVC]
        wqkv = np.ascontiguousarray(
            np.concatenate([wq, wk, wv], axis=1).astype(ml_dtypes.bfloat16)
        )
        bq = b_attn[s : s + QKVC]
        bk = b_attn[C + s : C + s + QKVC]
        bv = b_attn[2 * C + s : 2 * C + s + QKVC]
        bqkv = np.ascontiguousarray(np.concatenate([bq, bk, bv]))
        wproj = np.ascontiguousarray(W_proj[s : s + QKVC, :].astype(ml_dtypes.bfloat16))
        in_maps.append({"xt": xt, "wqkv": wqkv, "bqkv": bqkv, "wproj": wproj})
    return in_maps


def run(x, W_attn, b_attn, W_proj, b_proj, trace=False, **kwargs):
    nc = _get_nc()
    in_maps = make_in_maps(x, W_attn, b_attn, W_proj)
    res = run_bass_kernel_spmd(
        nc, in_maps, core_ids=list(range(NCORES)), trace=trace, **kwargs
    )
    acc = res.results[0]["outp"].astype(np.float32, copy=True)
    for c in range(1, NCORES):
        acc += res.results[c]["outp"].astype(np.float32)
    acc += np.asarray(b_proj, dtype=np.float32)[None, :]
    out = acc.reshape(B, T, C)
    return out, res


def kernel(x, W_attn, b_attn, W_proj, b_proj):
    out, _ = run(x, W_attn, b_attn, W_proj, b_proj, trace=False)
    return out
